# revision 9
# baseline (speedup 1.0000x reference)
"""Trainium2 Bass kernel for MoE expert gating (nn_ExpertGating).

Computes, for x [16384, 4096], gate_w [64, 4096], gate_b [64]:
    g = x @ gate_w.T + gate_b                 # [16384, 64] fp32 logits
    top_vals, top_idx = top_k(g, k=8)         # descending, ties -> lowest index
    expert_weights = softmax(top_vals, -1)    # [16384, 8]
returns (expert_weights fp32 [16384, 8], top_idx int32 [16384, 8]).

Sharding: data-parallel over tokens -- 2048 tokens per core on 8 cores, the
64x4096 gate weight replicated. Per core the kernel streams x^T (pre-transposed
on host so the contraction dim H lands on SBUF partitions) through the PE in
32 H-chunks, accumulating logits [64 experts, tokens] in PSUM with full-fp32
matmuls (needed: min gap between 8th/9th logit is ~8e-6, so bf16/fp32r variants
would flip indices). Tokens are processed in 4 phases of 512 so each phase's
top-k tail overlaps the next phase's matmuls. Top-8 uses the DVE's native
max/max_index instructions; softmax is exp/sum/reciprocal on ACT+DVE.

variant "ct2": the 32 H-chunks are split into two groups accumulated into the
two partition halves of one PSUM bank via PE column-tiling (tile_position
(0,0)/(0,64)), so two fp32 matmul streams run concurrently on the array;
halves are summed on the DVE afterwards.
"""

import numpy as np

import concourse.mybir as mybir
import concourse.tile as tile
from concourse import bacc
from concourse.bass_utils import run_bass_kernel_spmd

F32 = mybir.dt.float32
U32 = mybir.dt.uint32

NCORES = 8
T_FULL = 16384
H = 4096
E = 64
K = 8

T = T_FULL // NCORES          # 2048 tokens per core
NCHUNK = H // 128             # 32 contraction chunks
PHASES = 4
TP = T // PHASES              # 512 tokens per phase
TILES_P = TP // 128           # 4 token-tiles of 128 per phase
DMA_GROUPS = 4                # x DMAs per phase
CG = NCHUNK // DMA_GROUPS     # 8 chunks per DMA (2 MiB transfers)

VARIANT = "ct2"

_CACHE = {}


def _build(repeat=None, variant=VARIANT, dma_groups=DMA_GROUPS, xbufs=4):
    cg = NCHUNK // dma_groups  # chunks per x DMA
    nc = bacc.Bacc("TRN2", target_bir_lowering=False, debug=False,
                   num_devices=NCORES)
    xt = nc.dram_tensor("xt", [H, T], F32, kind="ExternalInput").ap()
    wt = nc.dram_tensor("wt", [H, E], F32, kind="ExternalInput").ap()
    bias = nc.dram_tensor("bias", [E, 1], F32, kind="ExternalInput").ap()
    ident = nc.dram_tensor("ident", [E, E], F32, kind="ExternalInput").ap()
    w_out = nc.dram_tensor("w_out", [T, K], F32, kind="ExternalOutput").ap()
    i_out = nc.dram_tensor("i_out", [T, K], U32, kind="ExternalOutput").ap()

    with tile.TileContext(nc) as tc:
        with (
            tc.tile_pool(name="const", bufs=1) as cpool,
            tc.tile_pool(name="x", bufs=xbufs) as xpool,
            tc.tile_pool(name="ps", bufs=2, space="PSUM") as pspool,
            tc.tile_pool(name="psT", bufs=2, space="PSUM") as psTpool,
            tc.tile_pool(name="lg", bufs=2) as lgpool,
            tc.tile_pool(name="sm", bufs=2) as smpool,
        ):
            wt_sb = cpool.tile([128, NCHUNK, E], F32)
            nc.sync.dma_start(wt_sb, wt.rearrange("(c p) e -> p c e", p=128))
            bias_sb = cpool.tile([E, 1], F32)
            nc.sync.dma_start(bias_sb, bias)
            ident_sb = cpool.tile([E, E], F32)
            nc.sync.dma_start(ident_sb, ident)

            def phase_matmul_base(p):
                """All 32 chunks accumulate into one [E, TP] PSUM half-use."""
                ps = pspool.tile([E, TP], F32)
                for g in range(dma_groups):
                    xt_sb = xpool.tile([128, cg, TP], F32)
                    src = xt[g * cg * 128:(g + 1) * cg * 128,
                             p * TP:(p + 1) * TP]
                    nc.sync.dma_start(
                        xt_sb, src.rearrange("(j q) t -> q j t", q=128))
                    for j in range(cg):
                        c = g * cg + j
                        nc.tensor.matmul(
                            ps,
                            lhsT=wt_sb[:, c, :],
                            rhs=xt_sb[:, j, :],
                            start=(c == 0),
                            stop=(c == NCHUNK - 1),
                        )
                logits_sb = lgpool.tile([E, TP], F32)
                nc.vector.tensor_scalar_add(logits_sb, ps, bias_sb)
                return logits_sb

            def tail_tokens(logits_sb, tok0, ntok):
                """topk + softmax + store for ntok tokens starting at tok0.

                logits_sb is [E, ntok] (experts on partitions)."""
                tiles = ntok // 128
                ltT = lgpool.tile([128, TILES_P, E], F32)
                for t in range(tiles):
                    psT = psTpool.tile([128, E], F32)
                    nc.tensor.transpose(
                        psT, logits_sb[:, t * 128:(t + 1) * 128], ident_sb)
                    nc.scalar.activation(
                        ltT[:, t, :], psT,
                        mybir.ActivationFunctionType.Copy)

                vals = smpool.tile([128, TILES_P, K], F32)
                idxs = smpool.tile([128, TILES_P, K], U32)
                for t in range(tiles):
                    nc.vector.max(out=vals[:, t, :], in_=ltT[:, t, :])
                    nc.vector.max_index(out=idxs[:, t, :],
                                        in_max=vals[:, t, :],
                                        in_values=ltT[:, t, :])

                # store indices as soon as they exist (scalar HWDGE ring),
                # softmax continues in parallel
                dst_i = i_out[tok0:tok0 + ntok, :].rearrange(
                    "(t q) k -> q t k", q=128)
                nc.scalar.dma_start(dst_i, idxs[:, 0:tiles, :])

                sh = smpool.tile([128, TILES_P, K], F32)
                nc.vector.tensor_sub(
                    sh[:, 0:tiles, :], vals[:, 0:tiles, :],
                    vals[:, 0:tiles, 0:1].to_broadcast([128, tiles, K]))
                ex = smpool.tile([128, TILES_P, K], F32)
                nc.scalar.activation(
                    ex[:, 0:tiles, :], sh[:, 0:tiles, :],
                    mybir.ActivationFunctionType.Exp)
                sums = smpool.tile([128, TILES_P, 1], F32)
                nc.vector.reduce_sum(sums[:, 0:tiles, :], ex[:, 0:tiles, :],
                                     axis=mybir.AxisListType.X)
                rcp = smpool.tile([128, TILES_P, 1], F32)
                nc.vector.reciprocal(rcp[:, 0:tiles, :], sums[:, 0:tiles, :])
                wts = smpool.tile([128, TILES_P, K], F32)
                nc.vector.tensor_mul(
                    wts[:, 0:tiles, :], ex[:, 0:tiles, :],
                    rcp[:, 0:tiles, :].to_broadcast([128, tiles, K]))
                dst_w = w_out[tok0:tok0 + ntok, :].rearrange(
                    "(t q) k -> q t k", q=128)
                nc.sync.dma_start(dst_w, wts[:, 0:tiles, :])

            def superphase_ct2(p, subsizes):
                """One 512-token superphase: x loaded in dma_groups transfers,
                matmuls split into sub-phases (token ranges) with interleaved
                even/odd chunk streams on PE column halves; each sub-phase's
                logits finish right after its last chunk group so its tail
                overlaps later work."""
                HC = NCHUNK // 2
                subs = []   # (tok_off_in_phase, ntok, psum tile)
                off = 0
                for i, sz in enumerate(subsizes):
                    ps_sub = pspool.tile([128, sz], F32, tag=f"ps_sub{i}")
                    subs.append((off, sz, ps_sub))
                    off += sz
                assert off == TP
                for g in range(dma_groups):
                    xt_sb = xpool.tile([128, cg, TP], F32)
                    src = xt[g * cg * 128:(g + 1) * cg * 128,
                             p * TP:(p + 1) * TP]
                    nc.sync.dma_start(
                        xt_sb, src.rearrange("(j q) t -> q j t", q=128))
                    for j in range(cg):
                        c = g * cg + j
                        ci, half = divmod(c, 2)
                        for (soff, sz, ps) in subs:
                            nc.tensor.matmul(
                                ps[half * E:(half + 1) * E, :],
                                lhsT=wt_sb[:, c, :],
                                rhs=xt_sb[:, j, soff:soff + sz],
                                start=(ci == 0),
                                stop=(ci == HC - 1),
                                tile_position=(0, half * E),
                            )
                for (soff, sz, ps) in subs:
                    half_a = lgpool.tile([E, TP], F32)
                    nc.vector.tensor_scalar_add(
                        half_a[:, 0:sz], ps[0:E, :], bias_sb)
                    logits_sb = lgpool.tile([E, TP], F32)
                    nc.vector.tensor_add(
                        logits_sb[:, 0:sz], half_a[:, 0:sz], ps[E:2 * E, :])
                    tail_tokens(logits_sb[:, 0:sz], p * TP + soff, sz)

            def body():
                if variant in ("dmaonly", "dmaonly2"):
                    for p in range(PHASES):
                        for g in range(dma_groups):
                            eng = (nc.scalar if variant == "dmaonly2"
                                   and (p * dma_groups + g) % 2 else nc.sync)
                            xt_sb = xpool.tile([128, cg, TP], F32)
                            src = xt[g * cg * 128:(g + 1) * cg * 128,
                                     p * TP:(p + 1) * TP]
                            eng.dma_start(
                                xt_sb, src.rearrange("(j q) t -> q j t", q=128))
                    return
                if variant in ("ct2", "mmonly"):
                    for p in range(PHASES):
                        if p < PHASES - 1:
                            subsizes = [TP]
                        else:
                            subsizes = [256, 128, 128]
                        if variant == "mmonly":
                            subsizes = [TP]
                        superphase_ct2(p, subsizes)
                    return
                # base variant
                for p in range(PHASES):
                    logits_sb = phase_matmul_base(p)
                    tail_tokens(logits_sb, p * TP, TP)

            if repeat:
                with tc.For_i(0, repeat, 1,
                              hint_engines=(mybir.EngineType.PE,
                                            mybir.EngineType.SP)):
                    body()
            else:
                body()

    nc.compile()
    return nc


def get_nc():
    if "nc" not in _CACHE:
        _CACHE["nc"] = _build()
    return _CACHE["nc"]


def make_in_maps(x, gate_w, gate_b):
    wt = np.ascontiguousarray(np.asarray(gate_w, dtype=np.float32).T)
    bias = np.asarray(gate_b, dtype=np.float32).reshape(E, 1).copy()
    ident = np.eye(E, dtype=np.float32)
    x = np.asarray(x, dtype=np.float32)
    in_maps = []
    for c in range(NCORES):
        xt = np.ascontiguousarray(x[c * T:(c + 1) * T].T)
        in_maps.append({"xt": xt, "wt": wt, "bias": bias, "ident": ident})
    return in_maps


def kernel(x, gate_w, gate_b):
    nc = get_nc()
    in_maps = make_in_maps(x, gate_w, gate_b)
    res = run_bass_kernel_spmd(nc, in_maps, core_ids=list(range(NCORES)))
    weights = np.concatenate(
        [res.results[c]["w_out"] for c in range(NCORES)], axis=0)
    idx = np.concatenate(
        [res.results[c]["i_out"] for c in range(NCORES)], axis=0)
    return weights, idx.astype(np.int32)


# revision 17
# speedup vs baseline: 1.0488x; 1.0488x over previous
"""Trainium2 Bass kernel for MoE expert gating (nn_ExpertGating).

Computes, for x [16384, 4096], gate_w [64, 4096], gate_b [64]:
    g = x @ gate_w.T + gate_b                 # [16384, 64] fp32 logits
    top_vals, top_idx = top_k(g, k=8)         # descending, ties -> lowest index
    expert_weights = softmax(top_vals, -1)    # [16384, 8]
returns (expert_weights fp32 [16384, 8], top_idx int32 [16384, 8]).

Sharding: data-parallel over tokens -- 2048 tokens per core on 8 cores, the
64x4096 gate weight replicated. Per core the kernel streams x^T (pre-transposed
on host so the contraction dim H lands on SBUF partitions) through the PE in
32 H-chunks, accumulating logits [64 experts, tokens] in PSUM with full-fp32
matmuls (needed: min gap between 8th/9th logit is ~8e-6, so bf16/fp32r variants
would flip indices). Tokens are processed in 4 phases of 512 so each phase's
top-k tail overlaps the next phase's matmuls. Top-8 uses the DVE's native
max/max_index instructions; softmax is exp/sum/reciprocal on ACT+DVE.

variant "ct2": the 32 H-chunks are split into two groups accumulated into the
two partition halves of one PSUM bank via PE column-tiling (tile_position
(0,0)/(0,64)), so two fp32 matmul streams run concurrently on the array;
halves are summed on the DVE afterwards.
"""

import os
import subprocess
import sys
import tempfile
import time

import numpy as np

import concourse.mybir as mybir
import concourse.tile as tile
from concourse import bacc
from concourse.bass_utils import run_bass_kernel_spmd

F32 = mybir.dt.float32
U32 = mybir.dt.uint32

NCORES = 8
T_FULL = 16384
H = 4096
E = 64
K = 8

T = T_FULL // NCORES          # 2048 tokens per core
NCHUNK = H // 128             # 32 contraction chunks
PHASES = 4
TP = T // PHASES              # 512 tokens per phase
TILES_P = TP // 128           # 4 token-tiles of 128 per phase
DMA_GROUPS = 8                # x DMAs per phase (1 MiB transfers)

VARIANT = "ct2"

_CACHE = {}


def _build(repeat=None, variant=VARIANT, dma_groups=DMA_GROUPS, xbufs=6):
    cg = NCHUNK // dma_groups  # chunks per x DMA
    nc = bacc.Bacc("TRN2", target_bir_lowering=False, debug=False,
                   num_devices=NCORES)
    xt = nc.dram_tensor("xt", [H, T], F32, kind="ExternalInput").ap()
    wt = nc.dram_tensor("wt", [H, E], F32, kind="ExternalInput").ap()
    bias = nc.dram_tensor("bias", [E, 1], F32, kind="ExternalInput").ap()
    ident = nc.dram_tensor("ident", [E, E], F32, kind="ExternalInput").ap()
    w_out = nc.dram_tensor("w_out", [T, K], F32, kind="ExternalOutput").ap()
    i_out = nc.dram_tensor("i_out", [T, K], U32, kind="ExternalOutput").ap()

    with tile.TileContext(nc) as tc:
        with (
            tc.tile_pool(name="const", bufs=1) as cpool,
            tc.tile_pool(name="x", bufs=xbufs) as xpool,
            tc.tile_pool(name="ps", bufs=2, space="PSUM") as pspool,
            tc.tile_pool(name="psT", bufs=2, space="PSUM") as psTpool,
            tc.tile_pool(name="lg", bufs=2) as lgpool,
            tc.tile_pool(name="sm", bufs=2) as smpool,
        ):
            # consts load on the scalar HWDGE ring so the first x transfer
            # starts immediately on the sync ring
            wt_sb = cpool.tile([128, NCHUNK, E], F32)
            nc.scalar.dma_start(wt_sb, wt.rearrange("(c p) e -> p c e", p=128))
            bias_sb = cpool.tile([E, 1], F32)
            nc.scalar.dma_start(bias_sb, bias)
            ident_sb = cpool.tile([E, E], F32)
            nc.scalar.dma_start(ident_sb, ident)

            def phase_matmul_base(p):
                """All 32 chunks accumulate into one [E, TP] PSUM half-use."""
                ps = pspool.tile([E, TP], F32)
                for g in range(dma_groups):
                    xt_sb = xpool.tile([128, cg, TP], F32)
                    src = xt[g * cg * 128:(g + 1) * cg * 128,
                             p * TP:(p + 1) * TP]
                    nc.sync.dma_start(
                        xt_sb, src.rearrange("(j q) t -> q j t", q=128))
                    for j in range(cg):
                        c = g * cg + j
                        nc.tensor.matmul(
                            ps,
                            lhsT=wt_sb[:, c, :],
                            rhs=xt_sb[:, j, :],
                            start=(c == 0),
                            stop=(c == NCHUNK - 1),
                        )
                logits_sb = lgpool.tile([E, TP], F32)
                nc.vector.tensor_scalar_add(logits_sb, ps, bias_sb)
                return logits_sb

            def tail_tokens(logits_sb, tok0, ntok):
                """topk + softmax + store for ntok tokens starting at tok0.

                logits_sb is [E, ntok] (experts on partitions)."""
                tiles = ntok // 128
                ltT = lgpool.tile([128, TILES_P, E], F32)
                for t in range(tiles):
                    psT = psTpool.tile([128, E], F32)
                    nc.tensor.transpose(
                        psT, logits_sb[:, t * 128:(t + 1) * 128], ident_sb)
                    nc.scalar.activation(
                        ltT[:, t, :], psT,
                        mybir.ActivationFunctionType.Copy)

                vals = smpool.tile([128, TILES_P, K], F32)
                idxs = smpool.tile([128, TILES_P, K], U32)
                for t in range(tiles):
                    nc.vector.max(out=vals[:, t, :], in_=ltT[:, t, :])
                    nc.vector.max_index(out=idxs[:, t, :],
                                        in_max=vals[:, t, :],
                                        in_values=ltT[:, t, :])

                # store indices as soon as they exist (scalar HWDGE ring),
                # softmax continues in parallel
                dst_i = i_out[tok0:tok0 + ntok, :].rearrange(
                    "(t q) k -> q t k", q=128)
                nc.scalar.dma_start(dst_i, idxs[:, 0:tiles, :])

                sh = smpool.tile([128, TILES_P, K], F32)
                nc.vector.tensor_sub(
                    sh[:, 0:tiles, :], vals[:, 0:tiles, :],
                    vals[:, 0:tiles, 0:1].to_broadcast([128, tiles, K]))
                ex = smpool.tile([128, TILES_P, K], F32)
                nc.scalar.activation(
                    ex[:, 0:tiles, :], sh[:, 0:tiles, :],
                    mybir.ActivationFunctionType.Exp)
                sums = smpool.tile([128, TILES_P, 1], F32)
                nc.vector.reduce_sum(sums[:, 0:tiles, :], ex[:, 0:tiles, :],
                                     axis=mybir.AxisListType.X)
                rcp = smpool.tile([128, TILES_P, 1], F32)
                nc.vector.reciprocal(rcp[:, 0:tiles, :], sums[:, 0:tiles, :])
                wts = smpool.tile([128, TILES_P, K], F32)
                nc.vector.tensor_mul(
                    wts[:, 0:tiles, :], ex[:, 0:tiles, :],
                    rcp[:, 0:tiles, :].to_broadcast([128, tiles, K]))
                dst_w = w_out[tok0:tok0 + ntok, :].rearrange(
                    "(t q) k -> q t k", q=128)
                nc.scalar.dma_start(dst_w, wts[:, 0:tiles, :])

            def superphase_ct2(p, subsizes):
                """One 512-token superphase: x loaded in dma_groups transfers,
                matmuls split into sub-phases (token ranges) with interleaved
                even/odd chunk streams on PE column halves; each sub-phase's
                logits finish right after its last chunk group so its tail
                overlaps later work."""
                HC = NCHUNK // 2
                subs = []   # (tok_off_in_phase, ntok, psum tile)
                off = 0
                for i, sz in enumerate(subsizes):
                    ps_sub = pspool.tile([128, sz], F32, tag=f"ps_sub{i}")
                    subs.append((off, sz, ps_sub))
                    off += sz
                assert off == TP
                for g in range(dma_groups):
                    xt_sb = xpool.tile([128, cg, TP], F32)
                    src = xt[g * cg * 128:(g + 1) * cg * 128,
                             p * TP:(p + 1) * TP]
                    nc.sync.dma_start(
                        xt_sb, src.rearrange("(j q) t -> q j t", q=128))
                    # emit in A/B pairs per sub-phase so adjacent matmuls sit
                    # on opposite column halves and overlap on the array
                    for jp in range(cg // 2):
                        c0 = g * cg + 2 * jp      # even chunk -> half A
                        ci = c0 // 2
                        for (soff, sz, ps) in subs:
                            for half in (0, 1):
                                nc.tensor.matmul(
                                    ps[half * E:(half + 1) * E, :],
                                    lhsT=wt_sb[:, c0 + half, :],
                                    rhs=xt_sb[:, 2 * jp + half,
                                              soff:soff + sz],
                                    start=(ci == 0),
                                    stop=(ci == HC - 1),
                                    tile_position=(0, half * E),
                                )
                for (soff, sz, ps) in subs:
                    half_a = lgpool.tile([E, TP], F32)
                    nc.vector.tensor_scalar_add(
                        half_a[:, 0:sz], ps[0:E, :], bias_sb)
                    logits_sb = lgpool.tile([E, TP], F32)
                    nc.vector.tensor_add(
                        logits_sb[:, 0:sz], half_a[:, 0:sz], ps[E:2 * E, :])
                    tail_tokens(logits_sb[:, 0:sz], p * TP + soff, sz)

            def body():
                if variant == "empty":
                    t0 = smpool.tile([128, 8], F32)
                    nc.vector.memset(t0, 0.0)
                    return
                if variant == "dmaonly_contig":
                    # chunk-major: each partition gets one fully contiguous
                    # row segment of length T*4/ngrp bytes
                    ngrp = dma_groups // 4
                    tpb = T // ngrp
                    for g in range(NCHUNK // 4 * ngrp):
                        r, pg = divmod(g, ngrp)
                        xt_sb = xpool.tile([128, 4, tpb], F32)
                        src = xt[r * 512:(r + 1) * 512,
                                 pg * tpb:(pg + 1) * tpb]
                        nc.sync.dma_start(
                            xt_sb, src.rearrange("(j q) t -> q j t", q=128))
                    return
                if variant in ("dmaonly", "dmaonly2"):
                    for p in range(PHASES):
                        for g in range(dma_groups):
                            eng = (nc.scalar if variant == "dmaonly2"
                                   and (p * dma_groups + g) % 2 else nc.sync)
                            xt_sb = xpool.tile([128, cg, TP], F32)
                            src = xt[g * cg * 128:(g + 1) * cg * 128,
                                     p * TP:(p + 1) * TP]
                            eng.dma_start(
                                xt_sb, src.rearrange("(j q) t -> q j t", q=128))
                    return
                if variant in ("ct2", "ct2_flat", "ct2_sub3", "mmonly"):
                    last = {"ct2": [TP - 128, 128],
                            "ct2_flat": [TP],
                            "ct2_sub3": [TP - 256, 128, 128],
                            "mmonly": [TP]}[variant]
                    for p in range(PHASES):
                        subsizes = [TP] if p < PHASES - 1 else last
                        superphase_ct2(p, subsizes)
                    return
                # base variant
                for p in range(PHASES):
                    logits_sb = phase_matmul_base(p)
                    tail_tokens(logits_sb, p * TP, TP)

            if repeat:
                with tc.For_i(0, repeat, 1,
                              hint_engines=(mybir.EngineType.PE,
                                            mybir.EngineType.SP)):
                    body()
            else:
                body()

    nc.compile()
    return nc


def get_nc():
    if "nc" not in _CACHE:
        _CACHE["nc"] = _build()
    return _CACHE["nc"]


def make_in_maps(x, gate_w, gate_b):
    wt = np.ascontiguousarray(np.asarray(gate_w, dtype=np.float32).T)
    bias = np.asarray(gate_b, dtype=np.float32).reshape(E, 1).copy()
    ident = np.eye(E, dtype=np.float32)
    x = np.asarray(x, dtype=np.float32)
    in_maps = []
    for c in range(NCORES):
        xt = np.ascontiguousarray(x[c * T:(c + 1) * T].T)
        in_maps.append({"xt": xt, "wt": wt, "bias": bias, "ident": ident})
    return in_maps


def _kernel_inproc(x, gate_w, gate_b, attempts=3):
    nc = get_nc()
    in_maps = make_in_maps(x, gate_w, gate_b)
    last = None
    for attempt in range(attempts):
        try:
            res = run_bass_kernel_spmd(nc, in_maps,
                                       core_ids=list(range(NCORES)))
            break
        except Exception as e:  # transient NRT/axon failures observed
            last = e
            try:
                import jax
                for name in ("clear_backends",):
                    fn = getattr(jax, name, None)
                    if fn is not None:
                        fn()
            except Exception:
                pass
            time.sleep(3.0)
    else:
        raise last
    weights = np.concatenate(
        [res.results[c]["w_out"] for c in range(NCORES)], axis=0)
    idx = np.concatenate(
        [res.results[c]["i_out"] for c in range(NCORES)], axis=0)
    return weights, idx.astype(np.int32)


def _kernel_subprocess(x, gate_w, gate_b):
    """Run in a fresh process -- recovers from a wedged accelerator backend."""
    with tempfile.TemporaryDirectory() as td:
        inp = os.path.join(td, "in.npz")
        outp = os.path.join(td, "out.npz")
        np.savez(inp, x=x, gate_w=gate_w, gate_b=gate_b)
        env = {**os.environ, "_EXPERT_GATING_SUBPROC": "1"}
        subprocess.run([sys.executable, os.path.abspath(__file__), inp, outp],
                       check=True, env=env, timeout=1800)
        out = np.load(outp)
        return out["weights"], out["idx"]


def kernel(x, gate_w, gate_b):
    x = np.ascontiguousarray(np.asarray(x, dtype=np.float32))
    gate_w = np.asarray(gate_w, dtype=np.float32)
    gate_b = np.asarray(gate_b, dtype=np.float32)
    if os.environ.get("_EXPERT_GATING_SUBPROC") == "1":
        return _kernel_inproc(x, gate_w, gate_b)
    try:
        return _kernel_inproc(x, gate_w, gate_b)
    except Exception:
        return _kernel_subprocess(x, gate_w, gate_b)


if __name__ == "__main__":
    _in, _out = sys.argv[1], sys.argv[2]
    _d = np.load(_in)
    _w, _i = kernel(_d["x"], _d["gate_w"], _d["gate_b"])
    np.savez(_out, weights=_w, idx=_i)


# revision 22
# speedup vs baseline: 1.0503x; 1.0015x over previous
"""Trainium2 Bass kernel for MoE expert gating (nn_ExpertGating).

Computes, for x [16384, 4096], gate_w [64, 4096], gate_b [64]:
    g = x @ gate_w.T + gate_b                 # [16384, 64] fp32 logits
    top_vals, top_idx = top_k(g, k=8)         # descending, ties -> lowest index
    expert_weights = softmax(top_vals, -1)    # [16384, 8]
returns (expert_weights fp32 [16384, 8], top_idx int32 [16384, 8]).

Sharding: data-parallel over tokens -- 2048 tokens per core on 8 cores, the
64x4096 gate weight replicated (per the data-parallel sharding hint; no
inter-core communication). Per core the kernel streams x^T (pre-transposed on
host so the contraction dim H lands on SBUF partitions) through the PE in 32
H-chunks of [128, tokens], accumulating logits [64 experts, tokens] in PSUM
with full-fp32 matmuls. Full fp32 is required: the min gap between the 8th and
9th ranked logit on these inputs is ~8e-6, so bf16/fp32r matmul noise
(~1e-4) would flip output indices, while fp32 accumulation noise (~2e-6)
cannot.

The kernel is HBM-bound (32 MiB of x per core ~= 99 us at the ~340 GB/s
per-core streaming rate; everything else hides under it):
- x loads are 1 MiB HWDGE transfers on the sync ring (measured best shape),
  6-deep buffered; gate weight/bias/identity load on the scalar ring.
- "ct2": even H-chunks accumulate into PSUM partitions 0-63 via PE column
  groups 0-1, odd chunks into partitions 64-127 via column groups 2-3
  (tile_position (0,0)/(0,64), A/B-interleaved emission). The two fp32
  matmul streams run concurrently on disjoint array column halves, halving
  PE busy time (measured 144 -> 114 us before other tuning); the halves are
  summed on the DVE afterwards together with the bias.
- Tokens run in 4 superphases of 512 so each superphase's top-k tail
  overlaps the next one's DMA/matmuls; the last superphase splits into
  384+128-token sub-phases and loads its final 4 chunks as 2-chunk
  transfers (fine_last) to minimize the serial tail after the last byte.
- Top-8 + indices use the DVE's native max / max_index instructions
  (descending order, lowest-index ties -- matches jax.lax.top_k exactly);
  softmax of the 8 selected logits runs batched per superphase (sub, Exp on
  ACT, reduce_sum, reciprocal, mul on DVE), and outputs DMA out per phase.

Measured on trn2 (8 cores concurrent, hardware repeat-loop wall-clock
deltas): ~107 us per core steady-state vs a ~99 us pure-DMA floor for the
same transfers; indices match the CPU fp32 reference exactly on the graded
inputs, weights rel err ~3e-7.
"""

import os
import subprocess
import sys
import tempfile
import time

import numpy as np

import concourse.mybir as mybir
import concourse.tile as tile
from concourse import bacc
from concourse.bass_utils import run_bass_kernel_spmd

F32 = mybir.dt.float32
U32 = mybir.dt.uint32

NCORES = 8
T_FULL = 16384
H = 4096
E = 64
K = 8

T = T_FULL // NCORES          # 2048 tokens per core
NCHUNK = H // 128             # 32 contraction chunks
PHASES = 4
TP = T // PHASES              # 512 tokens per phase
TILES_P = TP // 128           # 4 token-tiles of 128 per phase
DMA_GROUPS = 8                # x DMAs per phase (1 MiB transfers)

VARIANT = "ct2"

_CACHE = {}


def _build(repeat=None, variant=VARIANT, dma_groups=DMA_GROUPS, xbufs=6,
           fine_last=True):
    cg = NCHUNK // dma_groups  # chunks per x DMA
    nc = bacc.Bacc("TRN2", target_bir_lowering=False, debug=False,
                   num_devices=NCORES)
    xt = nc.dram_tensor("xt", [H, T], F32, kind="ExternalInput").ap()
    wt = nc.dram_tensor("wt", [H, E], F32, kind="ExternalInput").ap()
    bias = nc.dram_tensor("bias", [E, 1], F32, kind="ExternalInput").ap()
    ident = nc.dram_tensor("ident", [E, E], F32, kind="ExternalInput").ap()
    w_out = nc.dram_tensor("w_out", [T, K], F32, kind="ExternalOutput").ap()
    i_out = nc.dram_tensor("i_out", [T, K], U32, kind="ExternalOutput").ap()

    with tile.TileContext(nc) as tc:
        with (
            tc.tile_pool(name="const", bufs=1) as cpool,
            tc.tile_pool(name="x", bufs=xbufs) as xpool,
            tc.tile_pool(name="ps", bufs=2, space="PSUM") as pspool,
            tc.tile_pool(name="psT", bufs=2, space="PSUM") as psTpool,
            tc.tile_pool(name="lg", bufs=2) as lgpool,
            tc.tile_pool(name="sm", bufs=2) as smpool,
        ):
            # consts load on the scalar HWDGE ring so the first x transfer
            # starts immediately on the sync ring
            wt_sb = cpool.tile([128, NCHUNK, E], F32)
            nc.scalar.dma_start(wt_sb, wt.rearrange("(c p) e -> p c e", p=128))
            bias_sb = cpool.tile([E, 1], F32)
            nc.scalar.dma_start(bias_sb, bias)
            ident_sb = cpool.tile([E, E], F32)
            nc.scalar.dma_start(ident_sb, ident)

            def phase_matmul_base(p):
                """All 32 chunks accumulate into one [E, TP] PSUM half-use."""
                ps = pspool.tile([E, TP], F32)
                for g in range(dma_groups):
                    xt_sb = xpool.tile([128, cg, TP], F32)
                    src = xt[g * cg * 128:(g + 1) * cg * 128,
                             p * TP:(p + 1) * TP]
                    nc.sync.dma_start(
                        xt_sb, src.rearrange("(j q) t -> q j t", q=128))
                    for j in range(cg):
                        c = g * cg + j
                        nc.tensor.matmul(
                            ps,
                            lhsT=wt_sb[:, c, :],
                            rhs=xt_sb[:, j, :],
                            start=(c == 0),
                            stop=(c == NCHUNK - 1),
                        )
                logits_sb = lgpool.tile([E, TP], F32)
                nc.vector.tensor_scalar_add(logits_sb, ps, bias_sb)
                return logits_sb

            def tail_tokens(logits_sb, tok0, ntok):
                """topk + softmax + store for ntok tokens starting at tok0.

                logits_sb is [E, ntok] (experts on partitions)."""
                tiles = ntok // 128
                ltT = lgpool.tile([128, TILES_P, E], F32)
                for t in range(tiles):
                    psT = psTpool.tile([128, E], F32)
                    nc.tensor.transpose(
                        psT, logits_sb[:, t * 128:(t + 1) * 128], ident_sb)
                    nc.scalar.activation(
                        ltT[:, t, :], psT,
                        mybir.ActivationFunctionType.Copy)

                vals = smpool.tile([128, TILES_P, K], F32)
                idxs = smpool.tile([128, TILES_P, K], U32)
                for t in range(tiles):
                    nc.vector.max(out=vals[:, t, :], in_=ltT[:, t, :])
                    nc.vector.max_index(out=idxs[:, t, :],
                                        in_max=vals[:, t, :],
                                        in_values=ltT[:, t, :])

                # store indices as soon as they exist (scalar HWDGE ring),
                # softmax continues in parallel
                dst_i = i_out[tok0:tok0 + ntok, :].rearrange(
                    "(t q) k -> q t k", q=128)
                nc.scalar.dma_start(dst_i, idxs[:, 0:tiles, :])

                sh = smpool.tile([128, TILES_P, K], F32)
                nc.vector.tensor_sub(
                    sh[:, 0:tiles, :], vals[:, 0:tiles, :],
                    vals[:, 0:tiles, 0:1].to_broadcast([128, tiles, K]))
                ex = smpool.tile([128, TILES_P, K], F32)
                nc.scalar.activation(
                    ex[:, 0:tiles, :], sh[:, 0:tiles, :],
                    mybir.ActivationFunctionType.Exp)
                sums = smpool.tile([128, TILES_P, 1], F32)
                nc.vector.reduce_sum(sums[:, 0:tiles, :], ex[:, 0:tiles, :],
                                     axis=mybir.AxisListType.X)
                rcp = smpool.tile([128, TILES_P, 1], F32)
                nc.vector.reciprocal(rcp[:, 0:tiles, :], sums[:, 0:tiles, :])
                wts = smpool.tile([128, TILES_P, K], F32)
                nc.vector.tensor_mul(
                    wts[:, 0:tiles, :], ex[:, 0:tiles, :],
                    rcp[:, 0:tiles, :].to_broadcast([128, tiles, K]))
                dst_w = w_out[tok0:tok0 + ntok, :].rearrange(
                    "(t q) k -> q t k", q=128)
                (nc.scalar if variant == "ct2_wsc" else nc.sync).dma_start(
                    dst_w, wts[:, 0:tiles, :])

            def superphase_ct2(p, subsizes):
                """One 512-token superphase: x loaded in dma_groups transfers,
                matmuls split into sub-phases (token ranges) with interleaved
                even/odd chunk streams on PE column halves; each sub-phase's
                logits finish right after its last chunk group so its tail
                overlaps later work."""
                HC = NCHUNK // 2
                subs = []   # (tok_off_in_phase, ntok, psum tile)
                off = 0
                for i, sz in enumerate(subsizes):
                    ps_sub = pspool.tile([128, sz], F32, tag=f"ps_sub{i}")
                    subs.append((off, sz, ps_sub))
                    off += sz
                assert off == TP
                # chunk-count per transfer: optionally split the final
                # transfer finer so the last matmuls wait on fewer bytes
                if fine_last and p == PHASES - 1:
                    groups = [cg] * (dma_groups - 1) + [2] * (cg // 2)
                else:
                    groups = [cg] * dma_groups
                cbase = 0
                for gi, ng in enumerate(groups):
                    xt_sb = xpool.tile([128, cg, TP], F32, tag="xt_sb")
                    src = xt[cbase * 128:(cbase + ng) * 128,
                             p * TP:(p + 1) * TP]
                    nc.sync.dma_start(
                        xt_sb[:, 0:ng, :],
                        src.rearrange("(j q) t -> q j t", q=128))
                    # emit in A/B pairs per sub-phase so adjacent matmuls sit
                    # on opposite column halves and overlap on the array
                    for jp in range(ng // 2):
                        c0 = cbase + 2 * jp       # even chunk -> half A
                        ci = c0 // 2
                        for (soff, sz, ps) in subs:
                            for half in (0, 1):
                                nc.tensor.matmul(
                                    ps[half * E:(half + 1) * E, :],
                                    lhsT=wt_sb[:, c0 + half, :],
                                    rhs=xt_sb[:, 2 * jp + half,
                                              soff:soff + sz],
                                    start=(ci == 0),
                                    stop=(ci == HC - 1),
                                    tile_position=(0, half * E),
                                )
                    cbase += ng
                for (soff, sz, ps) in subs:
                    half_a = lgpool.tile([E, TP], F32)
                    nc.vector.tensor_scalar_add(
                        half_a[:, 0:sz], ps[0:E, :], bias_sb)
                    logits_sb = lgpool.tile([E, TP], F32)
                    nc.vector.tensor_add(
                        logits_sb[:, 0:sz], half_a[:, 0:sz], ps[E:2 * E, :])
                    tail_tokens(logits_sb[:, 0:sz], p * TP + soff, sz)

            def body():
                if variant == "empty":
                    t0 = smpool.tile([128, 8], F32)
                    nc.vector.memset(t0, 0.0)
                    return
                if variant == "dmaonly_contig":
                    # chunk-major: each partition gets one fully contiguous
                    # row segment of length T*4/ngrp bytes
                    ngrp = dma_groups // 4
                    tpb = T // ngrp
                    for g in range(NCHUNK // 4 * ngrp):
                        r, pg = divmod(g, ngrp)
                        xt_sb = xpool.tile([128, 4, tpb], F32)
                        src = xt[r * 512:(r + 1) * 512,
                                 pg * tpb:(pg + 1) * tpb]
                        nc.sync.dma_start(
                            xt_sb, src.rearrange("(j q) t -> q j t", q=128))
                    return
                if variant in ("dmaonly", "dmaonly2", "dmaonly_mix"):
                    for p in range(PHASES):
                        for g in range(dma_groups):
                            i_dma = p * dma_groups + g
                            if variant == "dmaonly2":
                                eng = nc.scalar if i_dma % 2 else nc.sync
                            elif variant == "dmaonly_mix":
                                eng = nc.gpsimd if i_dma % 2 else nc.sync
                            else:
                                eng = nc.sync
                            xt_sb = xpool.tile([128, cg, TP], F32)
                            src = xt[g * cg * 128:(g + 1) * cg * 128,
                                     p * TP:(p + 1) * TP]
                            eng.dma_start(
                                xt_sb, src.rearrange("(j q) t -> q j t", q=128))
                    return
                if variant in ("ct2", "ct2_wsc", "ct2_flat", "ct2_sub3", "mmonly"):
                    last = {"ct2": [TP - 128, 128],
                            "ct2_wsc": [TP - 128, 128],
                            "ct2_flat": [TP],
                            "ct2_sub3": [TP - 256, 128, 128],
                            "mmonly": [TP]}[variant]
                    for p in range(PHASES):
                        subsizes = [TP] if p < PHASES - 1 else last
                        superphase_ct2(p, subsizes)
                    return
                # base variant
                for p in range(PHASES):
                    logits_sb = phase_matmul_base(p)
                    tail_tokens(logits_sb, p * TP, TP)

            if repeat:
                with tc.For_i(0, repeat, 1,
                              hint_engines=(mybir.EngineType.PE,
                                            mybir.EngineType.SP)):
                    body()
            else:
                body()

    nc.compile()
    return nc


def get_nc():
    if "nc" not in _CACHE:
        _CACHE["nc"] = _build()
    return _CACHE["nc"]


def make_in_maps(x, gate_w, gate_b):
    wt = np.ascontiguousarray(np.asarray(gate_w, dtype=np.float32).T)
    bias = np.asarray(gate_b, dtype=np.float32).reshape(E, 1).copy()
    ident = np.eye(E, dtype=np.float32)
    x = np.asarray(x, dtype=np.float32)
    in_maps = []
    for c in range(NCORES):
        xt = np.ascontiguousarray(x[c * T:(c + 1) * T].T)
        in_maps.append({"xt": xt, "wt": wt, "bias": bias, "ident": ident})
    return in_maps


def _kernel_inproc(x, gate_w, gate_b, attempts=3):
    nc = get_nc()
    in_maps = make_in_maps(x, gate_w, gate_b)
    last = None
    for attempt in range(attempts):
        try:
            res = run_bass_kernel_spmd(nc, in_maps,
                                       core_ids=list(range(NCORES)))
            break
        except Exception as e:  # transient NRT/axon failures observed
            last = e
            try:
                import jax
                for name in ("clear_backends",):
                    fn = getattr(jax, name, None)
                    if fn is not None:
                        fn()
            except Exception:
                pass
            time.sleep(3.0)
    else:
        raise last
    weights = np.concatenate(
        [res.results[c]["w_out"] for c in range(NCORES)], axis=0)
    idx = np.concatenate(
        [res.results[c]["i_out"] for c in range(NCORES)], axis=0)
    return weights, idx.astype(np.int32)


def _kernel_subprocess(x, gate_w, gate_b):
    """Run in a fresh process -- recovers from a wedged accelerator backend."""
    with tempfile.TemporaryDirectory() as td:
        inp = os.path.join(td, "in.npz")
        outp = os.path.join(td, "out.npz")
        np.savez(inp, x=x, gate_w=gate_w, gate_b=gate_b)
        env = {**os.environ, "_EXPERT_GATING_SUBPROC": "1"}
        subprocess.run([sys.executable, os.path.abspath(__file__), inp, outp],
                       check=True, env=env, timeout=1800)
        out = np.load(outp)
        return out["weights"], out["idx"]


def kernel(x, gate_w, gate_b):
    x = np.ascontiguousarray(np.asarray(x, dtype=np.float32))
    gate_w = np.asarray(gate_w, dtype=np.float32)
    gate_b = np.asarray(gate_b, dtype=np.float32)
    if os.environ.get("_EXPERT_GATING_SUBPROC") == "1":
        return _kernel_inproc(x, gate_w, gate_b)
    try:
        return _kernel_inproc(x, gate_w, gate_b)
    except Exception:
        return _kernel_subprocess(x, gate_w, gate_b)


if __name__ == "__main__":
    _in, _out = sys.argv[1], sys.argv[2]
    _d = np.load(_in)
    _w, _i = kernel(_d["x"], _d["gate_w"], _d["gate_b"])
    np.savez(_out, weights=_w, idx=_i)


# revision 25
# speedup vs baseline: 1.0507x; 1.0003x over previous
"""Trainium2 Bass kernel for MoE expert gating (nn_ExpertGating).

Computes, for x [16384, 4096], gate_w [64, 4096], gate_b [64]:
    g = x @ gate_w.T + gate_b                 # [16384, 64] fp32 logits
    top_vals, top_idx = top_k(g, k=8)         # descending, ties -> lowest index
    expert_weights = softmax(top_vals, -1)    # [16384, 8]
returns (expert_weights fp32 [16384, 8], top_idx int32 [16384, 8]).

Sharding: data-parallel over tokens -- 2048 tokens per core on 8 cores, the
64x4096 gate weight replicated (per the data-parallel sharding hint; no
inter-core communication). Per core the kernel streams x^T (pre-transposed on
host so the contraction dim H lands on SBUF partitions) through the PE in 32
H-chunks of [128, tokens], accumulating logits [64 experts, tokens] in PSUM
with full-fp32 matmuls. Full fp32 is required: the min gap between the 8th and
9th ranked logit on these inputs is ~8e-6, so bf16/fp32r matmul noise
(~1e-4) would flip output indices, while fp32 accumulation noise (~2e-6)
cannot.

The kernel is HBM-bound (32 MiB of x per core ~= 99 us at the ~340 GB/s
per-core streaming rate; everything else hides under it):
- x loads are 1 MiB HWDGE transfers on the sync ring (measured best shape),
  6-deep buffered; gate weight/bias/identity load on the scalar ring.
- "ct2": even H-chunks accumulate into PSUM partitions 0-63 via PE column
  groups 0-1, odd chunks into partitions 64-127 via column groups 2-3
  (tile_position (0,0)/(0,64), A/B-interleaved emission). The two fp32
  matmul streams run concurrently on disjoint array column halves, halving
  PE busy time (measured 144 -> 114 us before other tuning); the halves are
  summed on the DVE afterwards together with the bias.
- Tokens run in 4 superphases of 512 so each superphase's top-k tail
  overlaps the next one's DMA/matmuls; the last superphase splits into
  384+128-token sub-phases and loads its final 4 chunks as 2-chunk
  transfers (fine_last) to minimize the serial tail after the last byte.
- Top-8 + indices use the DVE's native max / max_index instructions
  (descending order, lowest-index ties -- matches jax.lax.top_k exactly);
  softmax of the 8 selected logits runs batched per superphase (sub, Exp on
  ACT, reduce_sum, reciprocal, mul on DVE), and outputs DMA out per phase.

Measured on trn2 (8 cores concurrent, hardware repeat-loop wall-clock
deltas): ~107 us per core steady-state vs a ~99 us pure-DMA floor for the
same transfers; indices match the CPU fp32 reference exactly on the graded
inputs, weights rel err ~3e-7.
"""

import os
import subprocess
import sys
import tempfile
import time

import numpy as np

import concourse.mybir as mybir
import concourse.tile as tile
from concourse import bacc
from concourse.bass_utils import run_bass_kernel_spmd

F32 = mybir.dt.float32
U32 = mybir.dt.uint32

NCORES = 8
T_FULL = 16384
H = 4096
E = 64
K = 8

T = T_FULL // NCORES          # 2048 tokens per core
NCHUNK = H // 128             # 32 contraction chunks
PHASES = 4
TP = T // PHASES              # 512 tokens per phase
TILES_P = TP // 128           # 4 token-tiles of 128 per phase
DMA_GROUPS = 8                # x DMAs per phase (1 MiB transfers)

VARIANT = "ct2"

_CACHE = {}


def _build(repeat=None, variant=VARIANT, dma_groups=DMA_GROUPS, xbufs=6,
           fine_last=True):
    cg = NCHUNK // dma_groups  # chunks per x DMA
    nc = bacc.Bacc("TRN2", target_bir_lowering=False, debug=False,
                   num_devices=NCORES)
    xt = nc.dram_tensor("xt", [H, T], F32, kind="ExternalInput").ap()
    wt = nc.dram_tensor("wt", [H, E], F32, kind="ExternalInput").ap()
    bias = nc.dram_tensor("bias", [E, 1], F32, kind="ExternalInput").ap()
    ident = nc.dram_tensor("ident", [E, E], F32, kind="ExternalInput").ap()
    biasb = nc.dram_tensor("biasb", [128, E], F32, kind="ExternalInput").ap()
    w_out = nc.dram_tensor("w_out", [T, K], F32, kind="ExternalOutput").ap()
    i_out = nc.dram_tensor("i_out", [T, K], U32, kind="ExternalOutput").ap()

    with tile.TileContext(nc) as tc:
        with (
            tc.tile_pool(name="const", bufs=1) as cpool,
            tc.tile_pool(name="x", bufs=xbufs) as xpool,
            tc.tile_pool(name="ps", bufs=2, space="PSUM") as pspool,
            tc.tile_pool(name="psT", bufs=2, space="PSUM") as psTpool,
            tc.tile_pool(name="lg", bufs=2) as lgpool,
            tc.tile_pool(name="sm", bufs=2) as smpool,
        ):
            # consts load on the scalar HWDGE ring so the first x transfer
            # starts immediately on the sync ring
            wt_sb = cpool.tile([128, NCHUNK, E], F32)
            nc.scalar.dma_start(wt_sb, wt.rearrange("(c p) e -> p c e", p=128))
            bias_sb = cpool.tile([E, 1], F32)
            nc.scalar.dma_start(bias_sb, bias)
            ident_sb = cpool.tile([E, E], F32)
            nc.scalar.dma_start(ident_sb, ident)
            biasb_sb = cpool.tile([128, E], F32)
            nc.scalar.dma_start(biasb_sb, biasb)

            def phase_matmul_base(p):
                """All 32 chunks accumulate into one [E, TP] PSUM half-use."""
                ps = pspool.tile([E, TP], F32)
                for g in range(dma_groups):
                    xt_sb = xpool.tile([128, cg, TP], F32)
                    src = xt[g * cg * 128:(g + 1) * cg * 128,
                             p * TP:(p + 1) * TP]
                    nc.sync.dma_start(
                        xt_sb, src.rearrange("(j q) t -> q j t", q=128))
                    for j in range(cg):
                        c = g * cg + j
                        nc.tensor.matmul(
                            ps,
                            lhsT=wt_sb[:, c, :],
                            rhs=xt_sb[:, j, :],
                            start=(c == 0),
                            stop=(c == NCHUNK - 1),
                        )
                logits_sb = lgpool.tile([E, TP], F32)
                nc.vector.tensor_scalar_add(logits_sb, ps, bias_sb)
                return logits_sb

            def tail_tokens(logits_sb, tok0, ntok):
                """topk + softmax + store for ntok tokens starting at tok0.

                logits_sb is [E, ntok] (experts on partitions)."""
                tiles = ntok // 128
                ltT = lgpool.tile([128, TILES_P, E], F32)
                for t in range(tiles):
                    psT = psTpool.tile([128, E], F32)
                    nc.tensor.transpose(
                        psT, logits_sb[:, t * 128:(t + 1) * 128], ident_sb)
                    nc.scalar.activation(
                        ltT[:, t, :], psT,
                        mybir.ActivationFunctionType.Copy)

                vals = smpool.tile([128, TILES_P, K], F32)
                idxs = smpool.tile([128, TILES_P, K], U32)
                for t in range(tiles):
                    nc.vector.max(out=vals[:, t, :], in_=ltT[:, t, :])
                    nc.vector.max_index(out=idxs[:, t, :],
                                        in_max=vals[:, t, :],
                                        in_values=ltT[:, t, :])

                # store indices as soon as they exist (scalar HWDGE ring),
                # softmax continues in parallel
                dst_i = i_out[tok0:tok0 + ntok, :].rearrange(
                    "(t q) k -> q t k", q=128)
                nc.scalar.dma_start(dst_i, idxs[:, 0:tiles, :])

                sh = smpool.tile([128, TILES_P, K], F32)
                nc.vector.tensor_sub(
                    sh[:, 0:tiles, :], vals[:, 0:tiles, :],
                    vals[:, 0:tiles, 0:1].to_broadcast([128, tiles, K]))
                ex = smpool.tile([128, TILES_P, K], F32)
                nc.scalar.activation(
                    ex[:, 0:tiles, :], sh[:, 0:tiles, :],
                    mybir.ActivationFunctionType.Exp)
                sums = smpool.tile([128, TILES_P, 1], F32)
                nc.vector.reduce_sum(sums[:, 0:tiles, :], ex[:, 0:tiles, :],
                                     axis=mybir.AxisListType.X)
                rcp = smpool.tile([128, TILES_P, 1], F32)
                nc.vector.reciprocal(rcp[:, 0:tiles, :], sums[:, 0:tiles, :])
                wts = smpool.tile([128, TILES_P, K], F32)
                nc.vector.tensor_mul(
                    wts[:, 0:tiles, :], ex[:, 0:tiles, :],
                    rcp[:, 0:tiles, :].to_broadcast([128, tiles, K]))
                dst_w = w_out[tok0:tok0 + ntok, :].rearrange(
                    "(t q) k -> q t k", q=128)
                (nc.scalar if variant == "ct2_wsc" else nc.sync).dma_start(
                    dst_w, wts[:, 0:tiles, :])

            def tail_tokens_tmajor(ltB, tok0, ntok):
                """topk+softmax+store for one [128, E] token-major tile."""
                assert ntok == 128
                vals = smpool.tile([128, K], F32, tag="valsB")
                idxs = smpool.tile([128, K], U32, tag="idxsB")
                nc.vector.max(out=vals, in_=ltB)
                nc.vector.max_index(out=idxs, in_max=vals, in_values=ltB)
                dst_i = i_out[tok0:tok0 + ntok, :]
                nc.scalar.dma_start(dst_i, idxs)
                neg = smpool.tile([128, 1], F32, tag="negB")
                nc.vector.tensor_scalar_mul(neg, vals[:, 0:1], -1.0)
                ex = smpool.tile([128, K], F32, tag="exB")
                ssum = smpool.tile([128, 1], F32, tag="ssumB")
                nc.scalar.activation(
                    ex, vals, mybir.ActivationFunctionType.Exp,
                    bias=neg, scale=1.0, accum_out=ssum)
                rcp = smpool.tile([128, 1], F32, tag="rcpB")
                nc.vector.reciprocal(rcp, ssum)
                wts = smpool.tile([128, K], F32, tag="wtsB")
                nc.vector.tensor_scalar_mul(wts, ex, rcp)
                nc.sync.dma_start(w_out[tok0:tok0 + ntok, :], wts)

            def superphase_ct2(p, subsizes):
                """One 512-token superphase: x loaded in dma_groups transfers,
                matmuls split into sub-phases (token ranges) with interleaved
                even/odd chunk streams on PE column halves; each sub-phase's
                logits finish right after its last chunk group so its tail
                overlaps later work."""
                HC = NCHUNK // 2
                subs = []   # (tok_off_in_phase, ntok, psum tile)
                bsubs = []  # token-major subs: (tok_off, ntok, psum tile)
                off = 0
                for i, sz in enumerate(subsizes):
                    if sz < 0:  # token-major (orientation B), |sz| tokens
                        sz = -sz
                        for bt in range(sz // 128):
                            ps_b = pspool.tile([128, E], F32,
                                               tag=f"ps_b{bt}")
                            bsubs.append((off + bt * 128, 128, ps_b))
                        subs.append((off, sz, None))
                    else:
                        ps_sub = pspool.tile([128, sz], F32, tag=f"ps_sub{i}")
                        subs.append((off, sz, ps_sub))
                    off += sz
                assert off == TP
                # chunk-count per transfer: optionally split the final
                # transfer finer so the last matmuls wait on fewer bytes
                if fine_last and p == PHASES - 1:
                    groups = [cg] * (dma_groups - 1) + [2] * (cg // 2)
                else:
                    groups = [cg] * dma_groups
                cbase = 0
                for gi, ng in enumerate(groups):
                    xt_sb = xpool.tile([128, cg, TP], F32, tag="xt_sb")
                    src = xt[cbase * 128:(cbase + ng) * 128,
                             p * TP:(p + 1) * TP]
                    eng = nc.sync
                    if (variant == "ct2_mix4" and p < PHASES - 1
                            and gi % 4 == 2):
                        eng = nc.gpsimd
                    eng.dma_start(
                        xt_sb[:, 0:ng, :],
                        src.rearrange("(j q) t -> q j t", q=128))
                    # emit in A/B pairs per sub-phase so adjacent matmuls sit
                    # on opposite column halves and overlap on the array
                    for jp in range(ng // 2):
                        c0 = cbase + 2 * jp       # even chunk -> half A
                        ci = c0 // 2
                        for (soff, sz, ps) in subs:
                            if ps is None:
                                continue
                            for half in (0, 1):
                                nc.tensor.matmul(
                                    ps[half * E:(half + 1) * E, :],
                                    lhsT=wt_sb[:, c0 + half, :],
                                    rhs=xt_sb[:, 2 * jp + half,
                                              soff:soff + sz],
                                    start=(ci == 0),
                                    stop=(ci == HC - 1),
                                    tile_position=(0, half * E),
                                )
                    # token-major subs: x block is the stationary operand,
                    # wT chunk the moving one -> psum [tokens, experts]
                    for j in range(ng):
                        c = cbase + j
                        for (boff, bsz, ps_b) in bsubs:
                            nc.tensor.matmul(
                                ps_b,
                                lhsT=xt_sb[:, j, boff:boff + bsz],
                                rhs=wt_sb[:, c, :],
                                start=(c == 0),
                                stop=(c == NCHUNK - 1),
                            )
                    cbase += ng
                for (boff, bsz, ps_b) in bsubs:
                    ltB = lgpool.tile([128, E], F32, tag="ltB")
                    nc.vector.tensor_add(ltB, ps_b, biasb_sb)
                    tail_tokens_tmajor(ltB, p * TP + boff, bsz)
                for (soff, sz, ps) in subs:
                    if ps is None:
                        continue
                    half_a = lgpool.tile([E, TP], F32)
                    nc.vector.tensor_scalar_add(
                        half_a[:, 0:sz], ps[0:E, :], bias_sb)
                    logits_sb = lgpool.tile([E, TP], F32)
                    nc.vector.tensor_add(
                        logits_sb[:, 0:sz], half_a[:, 0:sz], ps[E:2 * E, :])
                    tail_tokens(logits_sb[:, 0:sz], p * TP + soff, sz)

            def body():
                if variant == "empty":
                    t0 = smpool.tile([128, 8], F32)
                    nc.vector.memset(t0, 0.0)
                    return
                if variant == "dmaonly_contig":
                    # chunk-major: each partition gets one fully contiguous
                    # row segment of length T*4/ngrp bytes
                    ngrp = dma_groups // 4
                    tpb = T // ngrp
                    for g in range(NCHUNK // 4 * ngrp):
                        r, pg = divmod(g, ngrp)
                        xt_sb = xpool.tile([128, 4, tpb], F32)
                        src = xt[r * 512:(r + 1) * 512,
                                 pg * tpb:(pg + 1) * tpb]
                        nc.sync.dma_start(
                            xt_sb, src.rearrange("(j q) t -> q j t", q=128))
                    return
                if variant in ("dmaonly", "dmaonly2", "dmaonly_mix"):
                    for p in range(PHASES):
                        for g in range(dma_groups):
                            i_dma = p * dma_groups + g
                            if variant == "dmaonly2":
                                eng = nc.scalar if i_dma % 2 else nc.sync
                            elif variant == "dmaonly_mix":
                                eng = nc.gpsimd if i_dma % 2 else nc.sync
                            else:
                                eng = nc.sync
                            xt_sb = xpool.tile([128, cg, TP], F32)
                            src = xt[g * cg * 128:(g + 1) * cg * 128,
                                     p * TP:(p + 1) * TP]
                            eng.dma_start(
                                xt_sb, src.rearrange("(j q) t -> q j t", q=128))
                    return
                if variant in ("ct2", "ct2_tb", "ct2_mix4", "ct2_wsc", "ct2_flat", "ct2_sub3", "mmonly"):
                    last = {"ct2": [TP - 128, 128],
                            "ct2_tb": [TP - 128, -128],
                            "ct2_mix4": [TP - 128, 128],
                            "ct2_wsc": [TP - 128, 128],
                            "ct2_flat": [TP],
                            "ct2_sub3": [TP - 256, 128, 128],
                            "mmonly": [TP]}[variant]
                    for p in range(PHASES):
                        subsizes = [TP] if p < PHASES - 1 else last
                        superphase_ct2(p, subsizes)
                    return
                # base variant
                for p in range(PHASES):
                    logits_sb = phase_matmul_base(p)
                    tail_tokens(logits_sb, p * TP, TP)

            if repeat:
                with tc.For_i(0, repeat, 1,
                              hint_engines=(mybir.EngineType.PE,
                                            mybir.EngineType.SP)):
                    body()
            else:
                body()

    nc.compile()
    return nc


def get_nc():
    if "nc" not in _CACHE:
        _CACHE["nc"] = _build()
    return _CACHE["nc"]


def make_in_maps(x, gate_w, gate_b):
    wt = np.ascontiguousarray(np.asarray(gate_w, dtype=np.float32).T)
    bias = np.asarray(gate_b, dtype=np.float32).reshape(E, 1).copy()
    ident = np.eye(E, dtype=np.float32)
    biasb = np.tile(np.asarray(gate_b, dtype=np.float32).reshape(1, E),
                    (128, 1))
    x = np.asarray(x, dtype=np.float32)
    in_maps = []
    for c in range(NCORES):
        xt = np.ascontiguousarray(x[c * T:(c + 1) * T].T)
        in_maps.append({"xt": xt, "wt": wt, "bias": bias, "ident": ident,
                        "biasb": biasb})
    return in_maps


def _kernel_inproc(x, gate_w, gate_b, attempts=3):
    nc = get_nc()
    in_maps = make_in_maps(x, gate_w, gate_b)
    last = None
    for attempt in range(attempts):
        try:
            res = run_bass_kernel_spmd(nc, in_maps,
                                       core_ids=list(range(NCORES)))
            break
        except Exception as e:  # transient NRT/axon failures observed
            last = e
            try:
                import jax
                for name in ("clear_backends",):
                    fn = getattr(jax, name, None)
                    if fn is not None:
                        fn()
            except Exception:
                pass
            time.sleep(3.0)
    else:
        raise last
    weights = np.concatenate(
        [res.results[c]["w_out"] for c in range(NCORES)], axis=0)
    idx = np.concatenate(
        [res.results[c]["i_out"] for c in range(NCORES)], axis=0)
    return weights, idx.astype(np.int32)


def _kernel_subprocess(x, gate_w, gate_b):
    """Run in a fresh process -- recovers from a wedged accelerator backend."""
    with tempfile.TemporaryDirectory() as td:
        inp = os.path.join(td, "in.npz")
        outp = os.path.join(td, "out.npz")
        np.savez(inp, x=x, gate_w=gate_w, gate_b=gate_b)
        env = {**os.environ, "_EXPERT_GATING_SUBPROC": "1"}
        subprocess.run([sys.executable, os.path.abspath(__file__), inp, outp],
                       check=True, env=env, timeout=1800)
        out = np.load(outp)
        return out["weights"], out["idx"]


def kernel(x, gate_w, gate_b):
    x = np.ascontiguousarray(np.asarray(x, dtype=np.float32))
    gate_w = np.asarray(gate_w, dtype=np.float32)
    gate_b = np.asarray(gate_b, dtype=np.float32)
    if os.environ.get("_EXPERT_GATING_SUBPROC") == "1":
        return _kernel_inproc(x, gate_w, gate_b)
    try:
        return _kernel_inproc(x, gate_w, gate_b)
    except Exception:
        return _kernel_subprocess(x, gate_w, gate_b)


if __name__ == "__main__":
    _in, _out = sys.argv[1], sys.argv[2]
    _d = np.load(_in)
    _w, _i = kernel(_d["x"], _d["gate_w"], _d["gate_b"])
    np.savez(_out, weights=_w, idx=_i)


# revision 28
# speedup vs baseline: 1.0523x; 1.0015x over previous
"""Trainium2 Bass kernel for MoE expert gating (nn_ExpertGating).

Computes, for x [16384, 4096], gate_w [64, 4096], gate_b [64]:
    g = x @ gate_w.T + gate_b                 # [16384, 64] fp32 logits
    top_vals, top_idx = top_k(g, k=8)         # descending, ties -> lowest index
    expert_weights = softmax(top_vals, -1)    # [16384, 8]
returns (expert_weights fp32 [16384, 8], top_idx int32 [16384, 8]).

Sharding: data-parallel over tokens -- 2048 tokens per core on 8 cores, the
64x4096 gate weight replicated (per the data-parallel sharding hint; no
inter-core communication). Per core the kernel streams x^T (pre-transposed on
host so the contraction dim H lands on SBUF partitions) through the PE in 32
H-chunks of [128, tokens], accumulating logits [64 experts, tokens] in PSUM
with full-fp32 matmuls. Full fp32 is required: the min gap between the 8th and
9th ranked logit on these inputs is ~8e-6, so bf16/fp32r matmul noise
(~1e-4) would flip output indices, while fp32 accumulation noise (~2e-6)
cannot.

The kernel is HBM-bound (32 MiB of x per core ~= 99 us at the ~340 GB/s
per-core streaming rate; everything else hides under it):
- x loads are 1 MiB HWDGE transfers on the sync ring (measured best shape),
  6-deep buffered; gate weight/bias/identity load on the scalar ring.
- "ct2": even H-chunks accumulate into PSUM partitions 0-63 via PE column
  groups 0-1, odd chunks into partitions 64-127 via column groups 2-3
  (tile_position (0,0)/(0,64), A/B-interleaved emission). The two fp32
  matmul streams run concurrently on disjoint array column halves, halving
  PE busy time (measured 144 -> 114 us before other tuning); the halves are
  summed on the DVE afterwards together with the bias.
- Tokens run in 4 superphases of 512 so each superphase's top-k tail
  overlaps the next one's DMA/matmuls; the last superphase splits into
  384+128-token sub-phases and loads its final 4 chunks as 2-chunk
  transfers (fine_last) to minimize the serial tail after the last byte.
- The per-phase index store is issued from the sync sequencer (the scalar
  sequencer would stall its own Exp dispatch behind the DMA descriptor
  generation; measured 109.8 vs 111.1 us in direct A/B).
- Top-8 + indices use the DVE's native max / max_index instructions
  (descending order, lowest-index ties -- matches jax.lax.top_k exactly);
  softmax of the 8 selected logits runs batched per superphase (sub, Exp on
  ACT, reduce_sum, reciprocal, mul on DVE), and outputs DMA out per phase.

Measured on trn2 (8 cores concurrent, hardware repeat-loop wall-clock
deltas): ~107 us per core steady-state vs a ~99 us pure-DMA floor for the
same transfers; indices match the CPU fp32 reference exactly on the graded
inputs, weights rel err ~3e-7.
"""

import os
import subprocess
import sys
import tempfile
import time

import numpy as np

import concourse.mybir as mybir
import concourse.tile as tile
from concourse import bacc
from concourse.bass_utils import run_bass_kernel_spmd

F32 = mybir.dt.float32
U32 = mybir.dt.uint32

NCORES = 8
T_FULL = 16384
H = 4096
E = 64
K = 8

T = T_FULL // NCORES          # 2048 tokens per core
NCHUNK = H // 128             # 32 contraction chunks
PHASES = 4
TP = T // PHASES              # 512 tokens per phase
TILES_P = TP // 128           # 4 token-tiles of 128 per phase
DMA_GROUPS = 8                # x DMAs per phase (1 MiB transfers)

VARIANT = "ct2_isync"

_CACHE = {}


def _build(repeat=None, variant=VARIANT, dma_groups=DMA_GROUPS, xbufs=6,
           fine_last=True):
    cg = NCHUNK // dma_groups  # chunks per x DMA
    nc = bacc.Bacc("TRN2", target_bir_lowering=False, debug=False,
                   num_devices=NCORES)
    xt = nc.dram_tensor("xt", [H, T], F32, kind="ExternalInput").ap()
    wt = nc.dram_tensor("wt", [H, E], F32, kind="ExternalInput").ap()
    bias = nc.dram_tensor("bias", [E, 1], F32, kind="ExternalInput").ap()
    ident = nc.dram_tensor("ident", [E, E], F32, kind="ExternalInput").ap()
    biasb = nc.dram_tensor("biasb", [128, E], F32, kind="ExternalInput").ap()
    w_out = nc.dram_tensor("w_out", [T, K], F32, kind="ExternalOutput").ap()
    i_out = nc.dram_tensor("i_out", [T, K], U32, kind="ExternalOutput").ap()

    with tile.TileContext(nc) as tc:
        with (
            tc.tile_pool(name="const", bufs=1) as cpool,
            tc.tile_pool(name="x", bufs=xbufs) as xpool,
            tc.tile_pool(name="ps", bufs=2, space="PSUM") as pspool,
            tc.tile_pool(name="psT", bufs=2, space="PSUM") as psTpool,
            tc.tile_pool(name="lg", bufs=2) as lgpool,
            tc.tile_pool(name="sm", bufs=2) as smpool,
        ):
            # consts load on the scalar HWDGE ring so the first x transfer
            # starts immediately on the sync ring
            wt_sb = cpool.tile([128, NCHUNK, E], F32)
            nc.scalar.dma_start(wt_sb, wt.rearrange("(c p) e -> p c e", p=128))
            bias_sb = cpool.tile([E, 1], F32)
            nc.scalar.dma_start(bias_sb, bias)
            ident_sb = cpool.tile([E, E], F32)
            nc.scalar.dma_start(ident_sb, ident)
            biasb_sb = cpool.tile([128, E], F32)
            nc.scalar.dma_start(biasb_sb, biasb)

            def phase_matmul_base(p):
                """All 32 chunks accumulate into one [E, TP] PSUM half-use."""
                ps = pspool.tile([E, TP], F32)
                for g in range(dma_groups):
                    xt_sb = xpool.tile([128, cg, TP], F32)
                    src = xt[g * cg * 128:(g + 1) * cg * 128,
                             p * TP:(p + 1) * TP]
                    nc.sync.dma_start(
                        xt_sb, src.rearrange("(j q) t -> q j t", q=128))
                    for j in range(cg):
                        c = g * cg + j
                        nc.tensor.matmul(
                            ps,
                            lhsT=wt_sb[:, c, :],
                            rhs=xt_sb[:, j, :],
                            start=(c == 0),
                            stop=(c == NCHUNK - 1),
                        )
                logits_sb = lgpool.tile([E, TP], F32)
                nc.vector.tensor_scalar_add(logits_sb, ps, bias_sb)
                return logits_sb

            def tail_tokens(logits_sb, tok0, ntok):
                """topk + softmax + store for ntok tokens starting at tok0.

                logits_sb is [E, ntok] (experts on partitions)."""
                tiles = ntok // 128
                ltT = lgpool.tile([128, TILES_P, E], F32)
                for t in range(tiles):
                    psT = psTpool.tile([128, E], F32)
                    nc.tensor.transpose(
                        psT, logits_sb[:, t * 128:(t + 1) * 128], ident_sb)
                    nc.scalar.activation(
                        ltT[:, t, :], psT,
                        mybir.ActivationFunctionType.Copy)

                vals = smpool.tile([128, TILES_P, K], F32)
                idxs = smpool.tile([128, TILES_P, K], U32)
                for t in range(tiles):
                    nc.vector.max(out=vals[:, t, :], in_=ltT[:, t, :])
                    nc.vector.max_index(out=idxs[:, t, :],
                                        in_max=vals[:, t, :],
                                        in_values=ltT[:, t, :])

                # store indices as soon as they exist (scalar HWDGE ring),
                # softmax continues in parallel
                dst_i = i_out[tok0:tok0 + ntok, :].rearrange(
                    "(t q) k -> q t k", q=128)
                (nc.sync if variant == "ct2_isync" else nc.scalar).dma_start(
                    dst_i, idxs[:, 0:tiles, :])

                if tiles == 1:
                    # fused path: Exp(bias=-max) with accumulated sum on ACT
                    neg = smpool.tile([128, 1], F32, tag="neg1")
                    nc.vector.tensor_scalar_mul(neg, vals[:, 0, 0:1], -1.0)
                    ex1 = smpool.tile([128, K], F32, tag="ex1")
                    ssum = smpool.tile([128, 1], F32, tag="ssum1")
                    nc.scalar.activation(
                        ex1, vals[:, 0, :], mybir.ActivationFunctionType.Exp,
                        bias=neg, scale=1.0, accum_out=ssum)
                    rcp1 = smpool.tile([128, 1], F32, tag="rcp1")
                    nc.vector.reciprocal(rcp1, ssum)
                    wts1 = smpool.tile([128, K], F32, tag="wts1")
                    nc.vector.tensor_scalar_mul(wts1, ex1, rcp1)
                    nc.sync.dma_start(w_out[tok0:tok0 + ntok, :], wts1)
                    return
                sh = smpool.tile([128, TILES_P, K], F32)
                nc.vector.tensor_sub(
                    sh[:, 0:tiles, :], vals[:, 0:tiles, :],
                    vals[:, 0:tiles, 0:1].to_broadcast([128, tiles, K]))
                ex = smpool.tile([128, TILES_P, K], F32)
                nc.scalar.activation(
                    ex[:, 0:tiles, :], sh[:, 0:tiles, :],
                    mybir.ActivationFunctionType.Exp)
                sums = smpool.tile([128, TILES_P, 1], F32)
                nc.vector.reduce_sum(sums[:, 0:tiles, :], ex[:, 0:tiles, :],
                                     axis=mybir.AxisListType.X)
                rcp = smpool.tile([128, TILES_P, 1], F32)
                nc.vector.reciprocal(rcp[:, 0:tiles, :], sums[:, 0:tiles, :])
                wts = smpool.tile([128, TILES_P, K], F32)
                nc.vector.tensor_mul(
                    wts[:, 0:tiles, :], ex[:, 0:tiles, :],
                    rcp[:, 0:tiles, :].to_broadcast([128, tiles, K]))
                dst_w = w_out[tok0:tok0 + ntok, :].rearrange(
                    "(t q) k -> q t k", q=128)
                (nc.scalar if variant == "ct2_wsc" else nc.sync).dma_start(
                    dst_w, wts[:, 0:tiles, :])

            def tail_tokens_tmajor(ltB, tok0, ntok):
                """topk+softmax+store for one [128, E] token-major tile."""
                assert ntok == 128
                vals = smpool.tile([128, K], F32, tag="valsB")
                idxs = smpool.tile([128, K], U32, tag="idxsB")
                nc.vector.max(out=vals, in_=ltB)
                nc.vector.max_index(out=idxs, in_max=vals, in_values=ltB)
                dst_i = i_out[tok0:tok0 + ntok, :]
                nc.scalar.dma_start(dst_i, idxs)
                neg = smpool.tile([128, 1], F32, tag="negB")
                nc.vector.tensor_scalar_mul(neg, vals[:, 0:1], -1.0)
                ex = smpool.tile([128, K], F32, tag="exB")
                ssum = smpool.tile([128, 1], F32, tag="ssumB")
                nc.scalar.activation(
                    ex, vals, mybir.ActivationFunctionType.Exp,
                    bias=neg, scale=1.0, accum_out=ssum)
                rcp = smpool.tile([128, 1], F32, tag="rcpB")
                nc.vector.reciprocal(rcp, ssum)
                wts = smpool.tile([128, K], F32, tag="wtsB")
                nc.vector.tensor_scalar_mul(wts, ex, rcp)
                nc.sync.dma_start(w_out[tok0:tok0 + ntok, :], wts)

            def superphase_ct2(p, subsizes):
                """One 512-token superphase: x loaded in dma_groups transfers,
                matmuls split into sub-phases (token ranges) with interleaved
                even/odd chunk streams on PE column halves; each sub-phase's
                logits finish right after its last chunk group so its tail
                overlaps later work."""
                HC = NCHUNK // 2
                subs = []   # (tok_off, ntok, mode, psum tile)
                bsubs = []  # token-major subs: (tok_off, ntok, psum tile)
                off = 0
                for i, spec in enumerate(subsizes):
                    sz, mode = spec if isinstance(spec, tuple) else (spec,
                                                                     "ab")
                    if sz < 0:  # token-major (orientation B), |sz| tokens
                        sz = -sz
                        for bt in range(sz // 128):
                            ps_b = pspool.tile([128, E], F32,
                                               tag=f"ps_b{bt}")
                            bsubs.append((off + bt * 128, 128, ps_b))
                        subs.append((off, sz, "b", None))
                    elif mode == "one":
                        # all 32 chunks on column half A -> psum [E, sz];
                        # tail needs only one bias-add, no halves-sum
                        ps_one = pspool.tile([E, sz], F32, tag=f"ps_one{i}")
                        subs.append((off, sz, "one", ps_one))
                    else:
                        ps_sub = pspool.tile([128, sz], F32, tag=f"ps_sub{i}")
                        subs.append((off, sz, "ab", ps_sub))
                    off += sz
                assert off == TP
                # chunk-count per transfer: optionally split the final
                # transfer finer so the last matmuls wait on fewer bytes
                if fine_last and p == PHASES - 1:
                    groups = [cg] * (dma_groups - 1) + [2] * (cg // 2)
                else:
                    groups = [cg] * dma_groups
                cbase = 0
                for gi, ng in enumerate(groups):
                    xt_sb = xpool.tile([128, cg, TP], F32, tag="xt_sb")
                    src = xt[cbase * 128:(cbase + ng) * 128,
                             p * TP:(p + 1) * TP]
                    eng = nc.sync
                    if (variant == "ct2_mix4" and p < PHASES - 1
                            and gi % 4 == 2):
                        eng = nc.gpsimd
                    eng.dma_start(
                        xt_sb[:, 0:ng, :],
                        src.rearrange("(j q) t -> q j t", q=128))
                    # emit in A/B pairs per sub-phase so adjacent matmuls sit
                    # on opposite column halves and overlap on the array
                    for jp in range(ng // 2):
                        c0 = cbase + 2 * jp       # even chunk -> half A
                        ci = c0 // 2
                        for (soff, sz, mode, ps) in subs:
                            if ps is None or mode != "ab":
                                continue
                            for half in (0, 1):
                                nc.tensor.matmul(
                                    ps[half * E:(half + 1) * E, :],
                                    lhsT=wt_sb[:, c0 + half, :],
                                    rhs=xt_sb[:, 2 * jp + half,
                                              soff:soff + sz],
                                    start=(ci == 0),
                                    stop=(ci == HC - 1),
                                    tile_position=(0, half * E),
                                )
                    # single-half subs: every chunk on column groups 0-1
                    for j in range(ng):
                        c = cbase + j
                        for (soff, sz, mode, ps) in subs:
                            if mode != "one":
                                continue
                            nc.tensor.matmul(
                                ps,
                                lhsT=wt_sb[:, c, :],
                                rhs=xt_sb[:, j, soff:soff + sz],
                                start=(c == 0),
                                stop=(c == NCHUNK - 1),
                                tile_position=(0, 0),
                            )
                    # token-major subs: x block is the stationary operand,
                    # wT chunk the moving one -> psum [tokens, experts]
                    for j in range(ng):
                        c = cbase + j
                        for (boff, bsz, ps_b) in bsubs:
                            nc.tensor.matmul(
                                ps_b,
                                lhsT=xt_sb[:, j, boff:boff + bsz],
                                rhs=wt_sb[:, c, :],
                                start=(c == 0),
                                stop=(c == NCHUNK - 1),
                            )
                    cbase += ng
                for (boff, bsz, ps_b) in bsubs:
                    ltB = lgpool.tile([128, E], F32, tag="ltB")
                    nc.vector.tensor_add(ltB, ps_b, biasb_sb)
                    tail_tokens_tmajor(ltB, p * TP + boff, bsz)
                for (soff, sz, mode, ps) in subs:
                    if ps is None:
                        continue
                    logits_sb = lgpool.tile([E, TP], F32)
                    if mode == "one":
                        nc.vector.tensor_scalar_add(
                            logits_sb[:, 0:sz], ps, bias_sb)
                    else:
                        half_a = lgpool.tile([E, TP], F32)
                        nc.vector.tensor_scalar_add(
                            half_a[:, 0:sz], ps[0:E, :], bias_sb)
                        nc.vector.tensor_add(
                            logits_sb[:, 0:sz], half_a[:, 0:sz],
                            ps[E:2 * E, :])
                    tail_tokens(logits_sb[:, 0:sz], p * TP + soff, sz)

            def body():
                if variant == "empty":
                    t0 = smpool.tile([128, 8], F32)
                    nc.vector.memset(t0, 0.0)
                    return
                if variant == "dmaonly_contig":
                    # chunk-major: each partition gets one fully contiguous
                    # row segment of length T*4/ngrp bytes
                    ngrp = dma_groups // 4
                    tpb = T // ngrp
                    for g in range(NCHUNK // 4 * ngrp):
                        r, pg = divmod(g, ngrp)
                        xt_sb = xpool.tile([128, 4, tpb], F32)
                        src = xt[r * 512:(r + 1) * 512,
                                 pg * tpb:(pg + 1) * tpb]
                        nc.sync.dma_start(
                            xt_sb, src.rearrange("(j q) t -> q j t", q=128))
                    return
                if variant in ("dmaonly", "dmaonly2", "dmaonly_mix"):
                    for p in range(PHASES):
                        for g in range(dma_groups):
                            i_dma = p * dma_groups + g
                            if variant == "dmaonly2":
                                eng = nc.scalar if i_dma % 2 else nc.sync
                            elif variant == "dmaonly_mix":
                                eng = nc.gpsimd if i_dma % 2 else nc.sync
                            else:
                                eng = nc.sync
                            xt_sb = xpool.tile([128, cg, TP], F32)
                            src = xt[g * cg * 128:(g + 1) * cg * 128,
                                     p * TP:(p + 1) * TP]
                            eng.dma_start(
                                xt_sb, src.rearrange("(j q) t -> q j t", q=128))
                    return
                if variant in ("ct2", "ct2_isync", "ct2_sf", "ct2_tb", "ct2_mix4", "ct2_wsc", "ct2_flat", "ct2_sub3", "mmonly"):
                    last = {"ct2": [TP - 128, 128],
                            "ct2_isync": [TP - 128, 128],
                            "ct2_sf": [TP - 128, (128, "one")],
                            "ct2_tb": [TP - 128, -128],
                            "ct2_mix4": [TP - 128, 128],
                            "ct2_wsc": [TP - 128, 128],
                            "ct2_flat": [TP],
                            "ct2_sub3": [TP - 256, 128, 128],
                            "mmonly": [TP]}[variant]
                    for p in range(PHASES):
                        subsizes = [TP] if p < PHASES - 1 else last
                        superphase_ct2(p, subsizes)
                    return
                # base variant
                for p in range(PHASES):
                    logits_sb = phase_matmul_base(p)
                    tail_tokens(logits_sb, p * TP, TP)

            if repeat:
                with tc.For_i(0, repeat, 1,
                              hint_engines=(mybir.EngineType.PE,
                                            mybir.EngineType.SP)):
                    body()
            else:
                body()

    nc.compile()
    return nc


def get_nc():
    if "nc" not in _CACHE:
        _CACHE["nc"] = _build()
    return _CACHE["nc"]


def make_in_maps(x, gate_w, gate_b):
    wt = np.ascontiguousarray(np.asarray(gate_w, dtype=np.float32).T)
    bias = np.asarray(gate_b, dtype=np.float32).reshape(E, 1).copy()
    ident = np.eye(E, dtype=np.float32)
    biasb = np.tile(np.asarray(gate_b, dtype=np.float32).reshape(1, E),
                    (128, 1))
    x = np.asarray(x, dtype=np.float32)
    in_maps = []
    for c in range(NCORES):
        xt = np.ascontiguousarray(x[c * T:(c + 1) * T].T)
        in_maps.append({"xt": xt, "wt": wt, "bias": bias, "ident": ident,
                        "biasb": biasb})
    return in_maps


def _kernel_inproc(x, gate_w, gate_b, attempts=3):
    nc = get_nc()
    in_maps = make_in_maps(x, gate_w, gate_b)
    last = None
    for attempt in range(attempts):
        try:
            res = run_bass_kernel_spmd(nc, in_maps,
                                       core_ids=list(range(NCORES)))
            break
        except Exception as e:  # transient NRT/axon failures observed
            last = e
            try:
                import jax
                for name in ("clear_backends",):
                    fn = getattr(jax, name, None)
                    if fn is not None:
                        fn()
            except Exception:
                pass
            time.sleep(3.0)
    else:
        raise last
    weights = np.concatenate(
        [res.results[c]["w_out"] for c in range(NCORES)], axis=0)
    idx = np.concatenate(
        [res.results[c]["i_out"] for c in range(NCORES)], axis=0)
    return weights, idx.astype(np.int32)


def _kernel_subprocess(x, gate_w, gate_b):
    """Run in a fresh process -- recovers from a wedged accelerator backend."""
    with tempfile.TemporaryDirectory() as td:
        inp = os.path.join(td, "in.npz")
        outp = os.path.join(td, "out.npz")
        np.savez(inp, x=x, gate_w=gate_w, gate_b=gate_b)
        env = {**os.environ, "_EXPERT_GATING_SUBPROC": "1"}
        subprocess.run([sys.executable, os.path.abspath(__file__), inp, outp],
                       check=True, env=env, timeout=1800)
        out = np.load(outp)
        return out["weights"], out["idx"]


def kernel(x, gate_w, gate_b):
    x = np.ascontiguousarray(np.asarray(x, dtype=np.float32))
    gate_w = np.asarray(gate_w, dtype=np.float32)
    gate_b = np.asarray(gate_b, dtype=np.float32)
    if os.environ.get("_EXPERT_GATING_SUBPROC") == "1":
        return _kernel_inproc(x, gate_w, gate_b)
    try:
        return _kernel_inproc(x, gate_w, gate_b)
    except Exception:
        return _kernel_subprocess(x, gate_w, gate_b)


if __name__ == "__main__":
    _in, _out = sys.argv[1], sys.argv[2]
    _d = np.load(_in)
    _w, _i = kernel(_d["x"], _d["gate_w"], _d["gate_b"])
    np.savez(_out, weights=_w, idx=_i)
